# revision 1
# baseline (speedup 1.0000x reference)
"""Trainium2 Bass kernel for nn_CapsuleLayer_31413390803000 (CapsuleLayer with
dynamic routing).

Mathematical collapse exploited
-------------------------------
The reference implements the classic CapsNet routing quirk: input_hat is the
same for every capsule k (the tf.matmul broadcast tiles W over k).  With b
initialised to zero:

  - iteration 0: softmax(0) = 1/16 exactly, so s[b,k,:] = colsum_n(h[b])/16,
    identical for all k; out rows are identical across k.
  - the agreement update b += h @ out^T is therefore constant along k, so
    softmax stays exactly uniform (exp(0)/16) for every later iteration.

Hence the whole 3-iteration routing reduces EXACTLY (bitwise in the
reference) to

  out[b, k, :] = squash( (sum_n inputs[b, n, :]) @ W / 16 )   for all k.

The kernel is therefore a memory-bound column-sum over N=1024 plus a tiny
[512] @ [512,256] matvec and the squash nonlinearity.

Sharding: data-parallel over batch B=16 across 8 cores (2 batches/core),
W replicated.  No cross-core communication.
"""

from contextlib import ExitStack

import numpy as np

import concourse.bass as bass  # noqa: F401
import concourse.tile as tile
from concourse import bacc, mybir
from concourse._compat import with_exitstack

# Problem shapes (hardcoded per contract).
B, N, DIN, KD = 16, 1024, 512, 256
NCAPS = 16
EPS = 1e-7
N_CORES = 8
B_LOC = B // N_CORES  # 2 batches per core

F32 = mybir.dt.float32
ACT = mybir.ActivationFunctionType


@with_exitstack
def _capsule_body(ctx: ExitStack, tc: "tile.TileContext", x, w, o, repeats: int = 1):
    """Per-core kernel body.

    x: [B_LOC, N, DIN] f32 DRAM in
    w: [DIN, KD]       f32 DRAM in
    o: [B_LOC, NCAPS, KD] f32 DRAM out
    repeats: re-execute the whole computation this many times (benchmarking
             only; results identical).
    """
    nc = tc.nc
    NCH = N // 128   # 8 n-chunks of 128 rows per batch
    DCH = DIN // 128  # 4 din-chunks
    CPD = 2          # n-chunks per DMA (512 KiB per dma_start)
    NFOLD = 3        # chunk pairs folded on GpSimd before the PE reduction
    # float32r would run the PE 4x faster but truncates mantissas
    # (measured ~1e-4..2.6e-4 rel err vs fp32's 6.6e-7); full fp32 keeps us
    # safely inside any fp32-envelope tolerance and the kernel stays
    # DMA-bound regardless.
    F32R = mybir.dt.float32

    xpool = ctx.enter_context(tc.tile_pool(name="xp", bufs=2 * B_LOC))
    wpool = ctx.enter_context(tc.tile_pool(name="wp", bufs=1))
    consts = ctx.enter_context(tc.tile_pool(name="cp", bufs=1))
    small = ctx.enter_context(tc.tile_pool(name="sp", bufs=2))
    outp = ctx.enter_context(tc.tile_pool(name="op", bufs=2))
    ps_v = ctx.enter_context(tc.tile_pool(name="ps_v", bufs=1, space="PSUM"))
    ps_t = ctx.enter_context(tc.tile_pool(name="ps_t", bufs=1, space="PSUM"))
    ps_y = ctx.enter_context(tc.tile_pool(name="ps_y", bufs=1, space="PSUM"))
    ps_o = ctx.enter_context(tc.tile_pool(name="ps_o", bufs=1, space="PSUM"))

    # Constants.  (The F32R indirection + producer-copy below keep the
    # kernel one switch away from float32r matmuls; with F32R = float32 the
    # extra copy is a single one-time [128,1] op.)
    ones_col_f = consts.tile([128, 1], F32)
    nc.vector.memset(ones_col_f, 1.0 / NCAPS)
    ones_col = consts.tile([128, 1], F32R)  # 1/16 folds the uniform softmax
    nc.vector.tensor_copy(ones_col, ones_col_f)
    one_one = consts.tile([1, 1], F32)
    nc.vector.memset(one_one, 1.0)
    ones_row = consts.tile([1, NCAPS], F32)
    nc.vector.memset(ones_row, 1.0)
    eps_t = consts.tile([1, 1], F32)
    nc.vector.memset(eps_t, EPS)

    # W [512,256] -> SBUF [128, 4, 256] (din-chunk c at free index c).
    # Loaded once (outside the repeat loop); sequenced on the SP ring after
    # batch 0's x chunks (see below) so batch 0's loads arrive first.
    w_r = wpool.tile([128, DCH, KD], F32R)

    # Prewarm the ACT sqrt table set so the ~2.7us LoadActFuncSet overlaps
    # the x DMAs instead of sitting in the squash critical path.
    warm = small.tile([1, 1], F32, tag="warm")
    nc.scalar.activation(warm, one_one, ACT.Sqrt, bias=eps_t)

    for _rep in range(repeats):
        # ---- issue all x-chunk DMAs (both batches), W between them ----
        xtiles = []
        for b in range(B_LOC):
            xr = x[b].rearrange("(a p) d -> p a d", p=128)  # [128, NCH, DIN]
            xt = xpool.tile([128, NCH, DIN], F32R, tag="x")
            # The last batch's final 512K chunk is split into two 256K
            # DMAs: the kernel's serial tail starts at the final chunk's
            # completion, and a half-size last transfer lands ~0.7us
            # earlier at the cost of one extra (fully overlapped)
            # descriptor generation.
            last_b = b == B_LOC - 1
            n_full = NCH // CPD - (1 if last_b else 0)
            for c in range(n_full):
                nc.sync.dma_start(
                    out=xt[:, c * CPD:(c + 1) * CPD, :],
                    in_=xr[:, c * CPD:(c + 1) * CPD, :].bitcast(F32R),
                )
            for a in range(n_full * CPD, NCH):
                nc.sync.dma_start(
                    out=xt[:, a:a + 1, :],
                    in_=xr[:, a:a + 1, :].bitcast(F32R),
                )
            xtiles.append(xt)
            if _rep == 0 and b == 0:
                nc.sync.dma_start(
                    out=w_r,
                    in_=w.rearrange("(c p) d -> p c d", p=128).bitcast(F32R),
                )

        v_pss = []
        for b in range(B_LOC):
            # ---- stage 1: v = (1/16) * colsum_n x[b]  ->  PSUM [1, DIN] ----
            # Chunks 0..5 fold pairwise on the (otherwise idle) GpSimd
            # engine — exact fp32 adds — so the PE only runs 5 fp32 matmuls
            # per batch instead of 8.  The last two chunks go straight to
            # the PE to keep the post-DMA critical path short.
            xt = xtiles[b]
            fold = xpool.tile([128, NFOLD, DIN], F32, tag="fold")
            for f in range(NFOLD):
                nc.gpsimd.tensor_add(
                    fold[:, f, :], xt[:, 2 * f, :], xt[:, 2 * f + 1, :]
                )
            rhs_list = [fold[:, f, :] for f in range(NFOLD)] + [
                xt[:, a, :] for a in range(2 * NFOLD, NCH)
            ]
            # v accumulates in TWO half-banks so the tail's PSUM->SBUF copy
            # can run on DVE and ACT in parallel (same-bank readers would
            # serialize; different banks don't).
            v_ph = [
                ps_v.tile([1, DIN // 2], F32, tag="v0", name=f"v0_{b}"),
                ps_v.tile([1, DIN // 2], F32, tag="v1", name=f"v1_{b}"),
            ]
            for i, rhs in enumerate(rhs_list):
                for h in (0, 1):
                    nc.tensor.matmul(
                        v_ph[h],
                        lhsT=ones_col,
                        rhs=rhs[:, h * (DIN // 2):(h + 1) * (DIN // 2)],
                        start=(i == 0),
                        stop=(i == len(rhs_list) - 1),
                    )
            v_pss.append(v_ph)

        # ---- stages 2-5 per batch, emitted after ALL stage-1 matmuls so
        # batch 0's tail (which ends in data-dependent PE work) doesn't
        # head-of-line-block batch 1's stage-1 in the PE FIFO. ----
        for b in range(B_LOC):
            v_ph = v_pss[b]
            v_sb = small.tile([1, DIN], F32, tag="v_sb")
            nc.vector.tensor_copy(v_sb[:, :DIN // 2], v_ph[0])
            nc.scalar.copy(v_sb[:, DIN // 2:], v_ph[1])

            # ---- stage 2: transpose v row -> 4 column chunks [128, 1] ----
            # matmul with a 1x1 identity rhs: out[m, 0] = v[0, m].  Plain
            # fp32 (fp32r has a dst-pattern restriction at N=1); these are
            # tiny N=1 matmuls.
            vt_sb = small.tile([128, DCH], F32R, tag="vt_sb")
            for dj in range(DCH):
                vt_ps = ps_t.tile([128, 1], F32, tag=f"vt{dj}")
                nc.tensor.matmul(
                    vt_ps,
                    lhsT=v_sb[:, dj * 128:(dj + 1) * 128],
                    rhs=one_one,
                    start=True,
                    stop=True,
                )
                nc.vector.tensor_copy(vt_sb[:, dj:dj + 1], vt_ps)

            # ---- stage 3: y = v @ W -> PSUM [1, KD] ----
            y_ps = ps_y.tile([1, KD], F32, tag="y")
            for dj in range(DCH):
                nc.tensor.matmul(
                    y_ps,
                    lhsT=vt_sb[:, dj:dj + 1],
                    rhs=w_r[:, dj, :],
                    start=(dj == 0),
                    stop=(dj == DCH - 1),
                )
            y_sb = small.tile([1, KD], F32, tag="y_sb")
            nc.vector.tensor_copy(y_sb, y_ps)

            # ---- stage 4: squash scale = sq / ((1+sq) * sqrt(sq+eps)) ----
            MUL, ADD = mybir.AluOpType.mult, mybir.AluOpType.add
            ysq = small.tile([1, KD], F32, tag="ysq")
            sq = small.tile([1, 1], F32, tag="sq")
            # ysq = y*y and sq = sum(y*y) in one DVE op (keeps ACT on the
            # sqrt table set only; an instruction may read only ONE
            # non-scalar input from PSUM, so this reads the SBUF copy).
            nc.vector.scalar_tensor_tensor(
                ysq, y_sb, 1.0, y_sb, op0=MUL, op1=MUL, accum_out=sq
            )
            s0 = small.tile([1, 1], F32, tag="s0")
            nc.scalar.activation(s0, sq, ACT.Sqrt, bias=eps_t)  # sqrt(sq+eps)
            # One Newton step tightens ACT sqrt: s1 = s0 + (sq+eps)/s0 is
            # 2*sqrt(sq+eps); the factor 2 is folded into sc below.
            rr = small.tile([1, 1], F32, tag="rr")
            nc.vector.reciprocal(rr, s0)
            t0 = small.tile([1, 1], F32, tag="t0")
            nc.vector.scalar_tensor_tensor(t0, sq, EPS, rr, op0=ADD, op1=MUL)
            s1 = small.tile([1, 1], F32, tag="s1")
            nc.vector.tensor_add(s1, s0, t0)  # 2*sqrt(sq+eps) refined
            d2 = small.tile([1, 1], F32, tag="d2")
            nc.vector.scalar_tensor_tensor(d2, sq, 1.0, s1, op0=ADD, op1=MUL)
            d3 = small.tile([1, 1], F32, tag="d3")
            nc.vector.reciprocal(d3, d2)  # 1 / (2*(1+sq)*sqrt(sq+eps))
            sc = small.tile([1, 1], F32, tag="sc")
            nc.vector.scalar_tensor_tensor(sc, sq, 2.0, d3, op0=MUL, op1=MUL)
            sc_row = small.tile([1, NCAPS], F32R, tag="sc_row")
            nc.vector.tensor_scalar_mul(sc_row, ones_row, sc)

            # ---- stage 5: out[k, :] = scale * y (broadcast to 16 caps) ----
            o_ps = ps_o.tile([NCAPS, KD], F32, tag="o")
            nc.tensor.matmul(
                o_ps,
                lhsT=sc_row,
                rhs=y_sb.bitcast(F32R),
                start=True,
                stop=True,
            )
            o_sb = outp.tile([NCAPS, KD], F32, tag="o_sb")
            nc.vector.tensor_copy(o_sb, o_ps)
            nc.scalar.dma_start(out=o[b], in_=o_sb)


def _build_nc(repeats: int = 1):
    nc = bacc.Bacc(
        "TRN2",
        target_bir_lowering=False,
        debug=False,
        num_devices=N_CORES,
    )
    x = nc.dram_tensor("x", [B_LOC, N, DIN], F32, kind="ExternalInput")
    w = nc.dram_tensor("w", [DIN, KD], F32, kind="ExternalInput")
    o = nc.dram_tensor("o", [B_LOC, NCAPS, KD], F32, kind="ExternalOutput")
    with tile.TileContext(nc) as tc:
        _capsule_body(tc, x.ap(), w.ap(), o.ap(), repeats=repeats)
    nc.compile()
    return nc


class Runner:
    """Cached PJRT executor for the SPMD bass kernel (8 cores).

    Mirrors concourse.bass2jax.run_bass_via_pjrt's multi-core path, but
    keeps the jitted executable alive so repeated kernel() calls don't
    re-trace/re-lower.
    """

    def __init__(self, repeats: int = 1):
        import jax
        from jax.experimental.shard_map import shard_map
        from jax.sharding import Mesh, PartitionSpec

        from concourse import bass2jax

        bass2jax.install_neuronx_cc_hook()
        self.nc = _build_nc(repeats=repeats)
        nc = self.nc

        partition_name = (
            nc.partition_id_tensor.name if nc.partition_id_tensor else None
        )
        in_names, out_names, out_avals, zero_outs = [], [], [], []
        for alloc in nc.m.functions[0].allocations:
            if not isinstance(alloc, mybir.MemoryLocationSet):
                continue
            name = alloc.memorylocations[0].name
            if alloc.kind == "ExternalInput":
                if name != partition_name:
                    in_names.append(name)
            elif alloc.kind == "ExternalOutput":
                shape = tuple(alloc.tensor_shape)
                dtype = mybir.dt.np(alloc.dtype)
                out_names.append(name)
                out_avals.append(jax.core.ShapedArray(shape, dtype))
                zero_outs.append(np.zeros(shape, dtype))
        self.in_names = in_names
        self.out_names = out_names
        self.out_avals = out_avals
        self.zero_outs = zero_outs
        n_params = len(in_names)
        n_outs = len(out_avals)
        all_in_names = in_names + out_names
        if partition_name is not None:
            all_in_names.append(partition_name)

        def _body(*args):
            operands = list(args)
            if partition_name is not None:
                operands.append(bass2jax.partition_id_tensor())
            outs = bass2jax._bass_exec_p.bind(
                *operands,
                out_avals=tuple(out_avals),
                in_names=tuple(all_in_names),
                out_names=tuple(out_names),
                lowering_input_output_aliases=(),
                sim_require_finite=True,
                sim_require_nnan=True,
                nc=nc,
            )
            return tuple(outs)

        self._body = _body
        devices = jax.devices()[:N_CORES]
        assert len(devices) == N_CORES
        self.mesh = Mesh(np.asarray(devices), ("core",))
        in_specs = (PartitionSpec("core"),) * (n_params + n_outs)
        out_specs = (PartitionSpec("core"),) * n_outs
        self.jitted = jax.jit(
            shard_map(
                _body,
                mesh=self.mesh,
                in_specs=in_specs,
                out_specs=out_specs,
                check_rep=False,
            ),
            donate_argnums=tuple(range(n_params, n_params + n_outs)),
            keep_unused=True,
        )

    def concat_inputs(self, in_maps):
        return [
            np.concatenate([np.asarray(m[name]) for m in in_maps], axis=0)
            for name in self.in_names
        ]

    def concat_zeros(self):
        return [
            np.zeros((N_CORES * z.shape[0], *z.shape[1:]), z.dtype)
            for z in self.zero_outs
        ]

    def __call__(self, concat_in):
        out_arrs = self.jitted(*concat_in, *self.concat_zeros())
        return [np.asarray(a) for a in out_arrs]


_RUNNERS: dict = {}


def get_runner(repeats: int = 1) -> Runner:
    if repeats not in _RUNNERS:
        _RUNNERS[repeats] = Runner(repeats=repeats)
    return _RUNNERS[repeats]


def kernel(inputs: np.ndarray, W: np.ndarray):
    """Full-input entry point: shard over 8 cores, run, gather."""
    assert inputs.shape == (B, N, DIN) and W.shape == (DIN, KD)
    runner = get_runner(1)
    xf = np.ascontiguousarray(inputs, dtype=np.float32)
    wf = np.ascontiguousarray(W, dtype=np.float32)
    in_maps = [
        {"x": xf[c * B_LOC:(c + 1) * B_LOC], "w": wf} for c in range(N_CORES)
    ]
    outs = runner(runner.concat_inputs(in_maps))
    # single output "o": [N_CORES * B_LOC, NCAPS, KD] -> [B, NCAPS, KD]
    return outs[0].reshape(B, NCAPS, KD)



# revision 2
# speedup vs baseline: 2.7724x; 2.7724x over previous
"""Trainium2 Bass kernel for nn_CapsuleLayer_31413390803000 (CapsuleLayer with
dynamic routing).

Mathematical collapse exploited
-------------------------------
The reference implements the classic CapsNet routing quirk: input_hat is the
same for every capsule k (the tf.matmul broadcast tiles W over k).  With b
initialised to zero:

  - iteration 0: softmax(0) = 1/16 exactly, so s[b,k,:] = colsum_n(h[b])/16,
    identical for all k; out rows are identical across k.
  - the agreement update b += h @ out^T is therefore constant along k, so
    softmax stays exactly uniform (exp(0)/16) for every later iteration.

Hence the whole 3-iteration routing reduces EXACTLY (bitwise in the
reference) to

  out[b, k, :] = squash( (sum_n inputs[b, n, :]) @ W / 16 )   for all k.

Kernel structure (per core: 2 batches, W replicated, no cross-core comms)
-------------------------------------------------------------------------
  front(rep): 8x 512 KiB HBM->SBUF DMAs (4 MiB of x), then ONE fp32r PSUM
    accumulation group computing v[2,512] = colsum/16 for BOTH batches at
    once (per-batch indicator columns as the stationary operand, so the
    other batch's row just accumulates +0).
  tail(rep):  v -> SBUF (DVE+ACT halves in parallel), transpose both
    batches via 4 matmuls against I2, y[2,256] = vT @ W, squash scale on
    DVE/ACT (off the PE critical path), scale applied to y BEFORE a
    constant 0/1 block-matmul broadcasts to the 16 capsules of both
    batches, one [32,256] copy, one 32 KiB output DMA.

Perf-critical choices (measured on HW, interleaved A/B):
  * float32r matmuls: 1 cycle/row vs fp32's 4 -> compute chain 12.7us ->
    3.8us.  rel err ~2.4e-4, far inside the 2e-2 envelope.
  * software-pipelined emission: tail(rep-1) is emitted AFTER front(rep),
    so the in-order PE queue never stalls on the tail's DVE/ACT round
    trips (engines execute in emission order; the tail is a latency chain).
  * no gpsimd pre-folding (it sat between DMA and PE and cost ~1-2us).
  * deep rings: 6 x-buffers (3 reps of DMA in flight), PSUM pools
    double-buffered and packed to exactly 8 banks (accumulation groups
    sharing a bank must be sequential, never interleaved).
  * squash Newton refinement dropped: ACT sqrt table precision is well
    inside the fp32r error floor anyway.

Steady state is DMA-bound at the ~2.3-2.6us/iter floor for the 4 MiB/core
x read (vs ~11-12us for the fp32 baseline in the same measurement epochs).
"""

from contextlib import ExitStack

import numpy as np

import concourse.bass as bass  # noqa: F401
import concourse.tile as tile
from concourse import bacc, mybir
from concourse._compat import with_exitstack

# Problem shapes (hardcoded per contract).
B, N, DIN, KD = 16, 1024, 512, 256
NCAPS = 16
EPS = 1e-7
N_CORES = 8
B_LOC = B // N_CORES  # 2 batches per core

F32 = mybir.dt.float32
F32R = mybir.dt.float32r
ACT = mybir.ActivationFunctionType


@with_exitstack
def _capsule_body(ctx: ExitStack, tc: "tile.TileContext", x, w, o,
                  repeats: int = 1):
    """Per-core kernel body.

    x: [B_LOC, N, DIN] f32 DRAM in
    w: [DIN, KD]       f32 DRAM in
    o: [B_LOC, NCAPS, KD] f32 DRAM out
    repeats: re-execute the whole computation this many times (benchmarking
             only; results identical).
    """
    nc = tc.nc
    NCH = N // 128   # 8 n-chunks of 128 rows per batch
    DCH = DIN // 128  # 4 din-chunks
    CPD = 2          # n-chunks per DMA (512 KiB per dma_start)
    MMT = F32R

    xpool = ctx.enter_context(tc.tile_pool(name="xp", bufs=6))
    wpool = ctx.enter_context(tc.tile_pool(name="wp", bufs=1))
    consts = ctx.enter_context(tc.tile_pool(name="cp", bufs=1))
    small = ctx.enter_context(tc.tile_pool(name="sp", bufs=3))
    outp = ctx.enter_context(tc.tile_pool(name="op", bufs=3))
    ps_v = ctx.enter_context(tc.tile_pool(name="ps_v", bufs=2, space="PSUM"))
    ps_t = ctx.enter_context(tc.tile_pool(name="ps_t", bufs=2, space="PSUM"))
    ps_y = ctx.enter_context(tc.tile_pool(name="ps_y", bufs=2, space="PSUM"))
    ps_o = ctx.enter_context(tc.tile_pool(name="ps_o", bufs=2, space="PSUM"))

    # ---- constants -------------------------------------------------------
    # per-batch indicator columns: ind[b] = [128, 2], col b = 1/16, other 0.
    # Lets ONE [2,512] PSUM accumulation group hold both batches' colsums
    # (engine partition access must start at 0/32/64/96, so separate [1,512]
    # rows at partitions 0 and 1 would be unreachable individually).
    ind_f = consts.tile([128, 2, 2], F32)
    nc.vector.memset(ind_f, 0.0)
    nc.vector.memset(ind_f[:, 0, 0:1], 1.0 / NCAPS)
    nc.vector.memset(ind_f[:, 1, 1:2], 1.0 / NCAPS)
    ind_r = consts.tile([128, 2, 2], MMT)
    nc.vector.tensor_copy(ind_r, ind_f)
    ind = [ind_r[:, 0, :], ind_r[:, 1, :]]

    one_one = consts.tile([1, 1], F32)
    nc.vector.memset(one_one, 1.0)
    IEQ, IGE = mybir.AluOpType.is_equal, mybir.AluOpType.is_ge
    ones22 = consts.tile([2, 2], F32)
    nc.vector.memset(ones22, 1.0)
    eye2 = consts.tile([2, 2], F32)
    nc.gpsimd.affine_select(  # iota[p, j] = p - j; keep where == 0
        eye2, ones22, pattern=[[-1, 2]], base=0, channel_multiplier=1,
        compare_op=IEQ, fill=0.0,
    )
    # block pattern [2, 32]: row b is 1.0 on cols b*16..b*16+15
    ones232 = consts.tile([2, 2 * NCAPS], F32)
    nc.vector.memset(ones232, 1.0)
    blk_t = consts.tile([2, 2 * NCAPS], F32)
    nc.gpsimd.affine_select(  # j - 16p >= 0
        blk_t, ones232, pattern=[[1, 2 * NCAPS]], base=0,
        channel_multiplier=-NCAPS, compare_op=IGE, fill=0.0,
    )
    blk_f = consts.tile([2, 2 * NCAPS], F32)
    nc.gpsimd.affine_select(  # 16p + 15 - j >= 0
        blk_f, blk_t, pattern=[[-1, 2 * NCAPS]], base=NCAPS - 1,
        channel_multiplier=NCAPS, compare_op=IGE, fill=0.0,
    )
    blk = consts.tile([2, 2 * NCAPS], MMT)
    nc.vector.tensor_copy(blk, blk_f)
    eps_t = consts.tile([1, 1], F32)
    nc.vector.memset(eps_t, EPS)
    eps2 = consts.tile([2, 1], F32)
    nc.vector.memset(eps2, EPS)

    # W [512,256] -> SBUF [128, 4, 256] (din-chunk c at free index c).
    w_r = wpool.tile([128, DCH, KD], MMT)

    # Prewarm the ACT sqrt table set so the ~2.7us LoadActFuncSet overlaps
    # the x DMAs instead of sitting in the squash critical path.
    warm = small.tile([1, 1], F32, tag="warm")
    nc.scalar.activation(warm, one_one, ACT.Sqrt, bias=eps_t)

    MUL = mybir.AluOpType.mult
    vstate = {}

    def emit_front(rep):
        # x DMAs + ONE stage-1 accumulation group for both batches.
        xtiles = []
        for b in range(B_LOC):
            xr = x[b].rearrange("(a p) d -> p a d", p=128)  # [128, NCH, DIN]
            xt = xpool.tile([128, NCH, DIN], MMT, tag="x")
            for c in range(NCH // CPD):
                nc.sync.dma_start(
                    out=xt[:, c * CPD:(c + 1) * CPD, :],
                    in_=xr[:, c * CPD:(c + 1) * CPD, :].bitcast(MMT),
                )
            xtiles.append(xt)
            if rep == 0 and b == 0:
                nc.sync.dma_start(
                    out=w_r,
                    in_=w.rearrange("(c p) d -> p c d", p=128).bitcast(MMT),
                )
        v_ps = ps_v.tile([2, DIN], F32, tag="v")
        for b in range(B_LOC):
            xt = xtiles[b]
            for a in range(NCH):
                nc.tensor.matmul(
                    v_ps,
                    lhsT=ind[b],
                    rhs=xt[:, a, :],
                    start=(b == 0 and a == 0),
                    stop=(b == B_LOC - 1 and a == NCH - 1),
                )
        vstate[rep] = v_ps

    def emit_tail(rep):
        v_ps = vstate.pop(rep)
        # v -> SBUF; free-dim halves on DVE + ACT in parallel
        v_sb = small.tile([2, DIN], F32, tag="v_sb")
        nc.vector.tensor_copy(v_sb[:, :DIN // 2], v_ps[:, :DIN // 2])
        nc.scalar.copy(v_sb[:, DIN // 2:], v_ps[:, DIN // 2:])

        # transpose both batches at once: vt[:, dj, b] = v[b, dj*128:...]
        vt_ps = ps_t.tile([128, DCH, 2], F32, tag="vt")
        for dj in range(DCH):
            nc.tensor.matmul(
                vt_ps[:, dj, :],
                lhsT=v_sb[:, dj * 128:(dj + 1) * 128],
                rhs=eye2,
                start=True,
                stop=True,
            )
        vt_sb = small.tile([128, DCH, 2], MMT, tag="vt_sb")
        nc.vector.tensor_copy(vt_sb, vt_ps)

        # y[b] = v[b] @ W for both batches in one accumulation group
        y_ps = ps_y.tile([2, KD], F32, tag="y")
        for dj in range(DCH):
            nc.tensor.matmul(
                y_ps,
                lhsT=vt_sb[:, dj, :],
                rhs=w_r[:, dj, :],
                start=(dj == 0),
                stop=(dj == DCH - 1),
            )
        y_sb = small.tile([2, KD], F32, tag="y_sb")
        nc.vector.tensor_copy(y_sb, y_ps)

        # squash scale per batch: sc = sq / ((1+sq) * sqrt(sq+eps)),
        # entirely on DVE/ACT; applied to y BEFORE the broadcast so the PE
        # broadcast matmul uses a constant lhsT and never waits on this.
        ysq = small.tile([2, KD], F32, tag="ysq")
        sq = small.tile([2, 1], F32, tag="sq")
        nc.vector.scalar_tensor_tensor(
            ysq, y_sb, 1.0, y_sb, op0=MUL, op1=MUL, accum_out=sq
        )
        s0 = small.tile([2, 1], F32, tag="s0")
        nc.scalar.activation(s0, sq, ACT.Sqrt, bias=eps2)
        d2 = small.tile([2, 1], F32, tag="d2")
        nc.vector.scalar_tensor_tensor(
            d2, sq, 1.0, s0, op0=mybir.AluOpType.add, op1=MUL
        )
        d3 = small.tile([2, 1], F32, tag="d3")
        nc.vector.reciprocal(d3, d2)
        sc = small.tile([2, 1], F32, tag="sc")
        nc.vector.tensor_mul(sc, sq, d3)
        ysc = small.tile([2, KD], MMT, tag="ysc")
        nc.vector.tensor_scalar_mul(ysc, y_sb, sc)

        # broadcast to 16 capsules per batch with the constant 0/1 block
        o_ps = ps_o.tile([2 * NCAPS, KD], F32, tag="o")
        nc.tensor.matmul(o_ps, lhsT=blk, rhs=ysc, start=True, stop=True)
        o_sb = outp.tile([2 * NCAPS, KD], F32, tag="o_sb")
        nc.scalar.copy(o_sb, o_ps)
        nc.scalar.dma_start(out=o.rearrange("b k d -> (b k) d"), in_=o_sb)

    # software pipeline: tail(rep-1) is emitted after front(rep) so the
    # in-order engine queues interleave the tail's latency chain with the
    # next rep's bulk work.
    for rep in range(repeats):
        emit_front(rep)
        if rep >= 1:
            emit_tail(rep - 1)
    emit_tail(repeats - 1)


def _build_nc(repeats: int = 1):
    nc = bacc.Bacc(
        "TRN2",
        target_bir_lowering=False,
        debug=False,
        num_devices=N_CORES,
    )
    x = nc.dram_tensor("x", [B_LOC, N, DIN], F32, kind="ExternalInput")
    w = nc.dram_tensor("w", [DIN, KD], F32, kind="ExternalInput")
    o = nc.dram_tensor("o", [B_LOC, NCAPS, KD], F32, kind="ExternalOutput")
    with tile.TileContext(nc) as tc:
        _capsule_body(tc, x.ap(), w.ap(), o.ap(), repeats=repeats)
    nc.compile()
    return nc


class Runner:
    """Cached PJRT executor for the SPMD bass kernel (8 cores).

    Mirrors concourse.bass2jax.run_bass_via_pjrt's multi-core path, but
    keeps the jitted executable alive so repeated kernel() calls don't
    re-trace/re-lower.
    """

    def __init__(self, repeats: int = 1):
        import jax
        from jax.experimental.shard_map import shard_map
        from jax.sharding import Mesh, PartitionSpec

        from concourse import bass2jax

        bass2jax.install_neuronx_cc_hook()
        self.nc = _build_nc(repeats=repeats)
        nc = self.nc

        partition_name = (
            nc.partition_id_tensor.name if nc.partition_id_tensor else None
        )
        in_names, out_names, out_avals, zero_outs = [], [], [], []
        for alloc in nc.m.functions[0].allocations:
            if not isinstance(alloc, mybir.MemoryLocationSet):
                continue
            name = alloc.memorylocations[0].name
            if alloc.kind == "ExternalInput":
                if name != partition_name:
                    in_names.append(name)
            elif alloc.kind == "ExternalOutput":
                shape = tuple(alloc.tensor_shape)
                dtype = mybir.dt.np(alloc.dtype)
                out_names.append(name)
                out_avals.append(jax.core.ShapedArray(shape, dtype))
                zero_outs.append(np.zeros(shape, dtype))
        self.in_names = in_names
        self.out_names = out_names
        self.out_avals = out_avals
        self.zero_outs = zero_outs
        n_params = len(in_names)
        n_outs = len(out_avals)
        all_in_names = in_names + out_names
        if partition_name is not None:
            all_in_names.append(partition_name)

        def _body(*args):
            operands = list(args)
            if partition_name is not None:
                operands.append(bass2jax.partition_id_tensor())
            outs = bass2jax._bass_exec_p.bind(
                *operands,
                out_avals=tuple(out_avals),
                in_names=tuple(all_in_names),
                out_names=tuple(out_names),
                lowering_input_output_aliases=(),
                sim_require_finite=True,
                sim_require_nnan=True,
                nc=nc,
            )
            return tuple(outs)

        self._body = _body
        devices = jax.devices()[:N_CORES]
        assert len(devices) == N_CORES
        self.mesh = Mesh(np.asarray(devices), ("core",))
        in_specs = (PartitionSpec("core"),) * (n_params + n_outs)
        out_specs = (PartitionSpec("core"),) * n_outs
        self.jitted = jax.jit(
            shard_map(
                _body,
                mesh=self.mesh,
                in_specs=in_specs,
                out_specs=out_specs,
                check_rep=False,
            ),
            donate_argnums=tuple(range(n_params, n_params + n_outs)),
            keep_unused=True,
        )

    def concat_inputs(self, in_maps):
        return [
            np.concatenate([np.asarray(m[name]) for m in in_maps], axis=0)
            for name in self.in_names
        ]

    def concat_zeros(self):
        return [
            np.zeros((N_CORES * z.shape[0], *z.shape[1:]), z.dtype)
            for z in self.zero_outs
        ]

    def __call__(self, concat_in):
        out_arrs = self.jitted(*concat_in, *self.concat_zeros())
        return [np.asarray(a) for a in out_arrs]


_RUNNERS: dict = {}


def get_runner(repeats: int = 1) -> Runner:
    if repeats not in _RUNNERS:
        _RUNNERS[repeats] = Runner(repeats=repeats)
    return _RUNNERS[repeats]


def kernel(inputs: np.ndarray, W: np.ndarray):
    """Full-input entry point: shard over 8 cores, run, gather."""
    assert inputs.shape == (B, N, DIN) and W.shape == (DIN, KD)
    runner = get_runner(1)
    xf = np.ascontiguousarray(inputs, dtype=np.float32)
    wf = np.ascontiguousarray(W, dtype=np.float32)
    in_maps = [
        {"x": xf[c * B_LOC:(c + 1) * B_LOC], "w": wf} for c in range(N_CORES)
    ]
    outs = runner(runner.concat_inputs(in_maps))
    # single output "o": [N_CORES * B_LOC, NCAPS, KD] -> [B, NCAPS, KD]
    return outs[0].reshape(B, NCAPS, KD)


# revision 4
# speedup vs baseline: 3.2222x; 1.1622x over previous
"""Trainium2 Bass kernel for nn_CapsuleLayer_31413390803000 (CapsuleLayer with
dynamic routing).

Mathematical collapse exploited
-------------------------------
The reference implements the classic CapsNet routing quirk: input_hat is the
same for every capsule k (the tf.matmul broadcast tiles W over k).  With b
initialised to zero:

  - iteration 0: softmax(0) = 1/16 exactly, so s[b,k,:] = colsum_n(h[b])/16,
    identical for all k; out rows are identical across k.
  - the agreement update b += h @ out^T is therefore constant along k, so
    softmax stays exactly uniform (exp(0)/16) for every later iteration.

Hence the whole 3-iteration routing reduces EXACTLY (bitwise in the
reference) to

  out[b, k, :] = squash( (sum_n inputs[b, n, :]) @ W / 16 )   for all k.

Kernel structure (per core: 2 batches, W replicated, no cross-core comms)
-------------------------------------------------------------------------
  front(rep): 8x 512 KiB HBM->SBUF DMAs (4 MiB of x), then ONE fp32r PSUM
    accumulation group computing v[2,512] = colsum/16 for BOTH batches at
    once (per-batch indicator columns as the stationary operand, so the
    other batch's row just accumulates +0).
  tail(rep):  v -> SBUF (DVE+ACT halves in parallel), transpose both
    batches via 4 matmuls against I2, y[2,256] = vT @ W, squash scale on
    DVE/ACT (off the PE critical path), scale applied to y BEFORE a
    constant 0/1 block-matmul broadcasts to the 16 capsules of both
    batches, one [32,256] copy, one 32 KiB output DMA.

Perf-critical choices (measured on HW, interleaved A/B):
  * float32r matmuls: 1 cycle/row vs fp32's 4 -> compute chain 12.7us ->
    3.8us.  rel err ~2.4e-4, far inside the 2e-2 envelope.
  * software-pipelined emission: tail(rep-1) is emitted AFTER front(rep),
    so the in-order PE queue never stalls on the tail's DVE/ACT round
    trips (engines execute in emission order; the tail is a latency chain).
  * no gpsimd pre-folding (it sat between DMA and PE and cost ~1-2us).
  * deep rings: 6 x-buffers (3 reps of DMA in flight), PSUM pools
    double-buffered and packed to exactly 8 banks (accumulation groups
    sharing a bank must be sequential, never interleaved).
  * squash Newton refinement dropped: ACT sqrt table precision is well
    inside the fp32r error floor anyway.

Steady state is DMA-bound at the ~2.3-2.6us/iter floor for the 4 MiB/core
x read (vs ~11-12us for the fp32 baseline in the same measurement epochs).
"""

from contextlib import ExitStack

import numpy as np

import concourse.bass as bass  # noqa: F401
import concourse.tile as tile
from concourse import bacc, mybir
from concourse._compat import with_exitstack

# Problem shapes (hardcoded per contract).
B, N, DIN, KD = 16, 1024, 512, 256
NCAPS = 16
EPS = 1e-7
N_CORES = 8
B_LOC = B // N_CORES  # 2 batches per core

F32 = mybir.dt.float32
F32R = mybir.dt.float32r
ACT = mybir.ActivationFunctionType


@with_exitstack
def _capsule_body(ctx: ExitStack, tc: "tile.TileContext", x, w, o,
                  repeats: int = 1):
    """Per-core kernel body.

    x: [B_LOC, N, DIN] f32 DRAM in
    w: [DIN, KD]       f32 DRAM in
    o: [B_LOC, NCAPS, KD] f32 DRAM out
    repeats: re-execute the whole computation this many times (benchmarking
             only; results identical).
    """
    nc = tc.nc
    NCH = N // 128   # 8 n-chunks of 128 rows per batch
    DCH = DIN // 128  # 4 din-chunks
    CPD = 2          # n-chunks per DMA (512 KiB per dma_start)
    MMT = F32R

    xpool = ctx.enter_context(tc.tile_pool(name="xp", bufs=6))
    fpool = ctx.enter_context(tc.tile_pool(name="fp", bufs=3))
    wpool = ctx.enter_context(tc.tile_pool(name="wp", bufs=1))
    consts = ctx.enter_context(tc.tile_pool(name="cp", bufs=1))
    small = ctx.enter_context(tc.tile_pool(name="sp", bufs=3))
    outp = ctx.enter_context(tc.tile_pool(name="op", bufs=3))
    ps_v = ctx.enter_context(tc.tile_pool(name="ps_v", bufs=2, space="PSUM"))
    ps_t = ctx.enter_context(tc.tile_pool(name="ps_t", bufs=2, space="PSUM"))
    ps_y = ctx.enter_context(tc.tile_pool(name="ps_y", bufs=2, space="PSUM"))
    ps_o = ctx.enter_context(tc.tile_pool(name="ps_o", bufs=2, space="PSUM"))

    # ---- constants -------------------------------------------------------
    # per-batch indicator columns: ind[b] = [128, 2], col b = 1/16, other 0.
    # Lets ONE [2,512] PSUM accumulation group hold both batches' colsums
    # (engine partition access must start at 0/32/64/96, so separate [1,512]
    # rows at partitions 0 and 1 would be unreachable individually).
    ind_f = consts.tile([128, 2, 2], F32)
    nc.vector.memset(ind_f, 0.0)
    nc.vector.memset(ind_f[:, 0, 0:1], 1.0 / NCAPS)
    nc.vector.memset(ind_f[:, 1, 1:2], 1.0 / NCAPS)
    ind_r = consts.tile([128, 2, 2], MMT)
    nc.vector.tensor_copy(ind_r, ind_f)
    ind = [ind_r[:, 0, :], ind_r[:, 1, :]]

    one_one = consts.tile([1, 1], F32)
    nc.vector.memset(one_one, 1.0)
    IEQ, IGE = mybir.AluOpType.is_equal, mybir.AluOpType.is_ge
    ones22 = consts.tile([2, 2], F32)
    nc.vector.memset(ones22, 1.0)
    eye2 = consts.tile([2, 2], F32)
    nc.gpsimd.affine_select(  # iota[p, j] = p - j; keep where == 0
        eye2, ones22, pattern=[[-1, 2]], base=0, channel_multiplier=1,
        compare_op=IEQ, fill=0.0,
    )
    # block pattern [2, 32]: row b is 1.0 on cols b*16..b*16+15
    ones232 = consts.tile([2, 2 * NCAPS], F32)
    nc.vector.memset(ones232, 1.0)
    blk_t = consts.tile([2, 2 * NCAPS], F32)
    nc.gpsimd.affine_select(  # j - 16p >= 0
        blk_t, ones232, pattern=[[1, 2 * NCAPS]], base=0,
        channel_multiplier=-NCAPS, compare_op=IGE, fill=0.0,
    )
    blk_f = consts.tile([2, 2 * NCAPS], F32)
    nc.gpsimd.affine_select(  # 16p + 15 - j >= 0
        blk_f, blk_t, pattern=[[-1, 2 * NCAPS]], base=NCAPS - 1,
        channel_multiplier=NCAPS, compare_op=IGE, fill=0.0,
    )
    blk = consts.tile([2, 2 * NCAPS], MMT)
    nc.vector.tensor_copy(blk, blk_f)
    eps_t = consts.tile([1, 1], F32)
    nc.vector.memset(eps_t, EPS)
    eps2 = consts.tile([2, 1], F32)
    nc.vector.memset(eps2, EPS)

    # W [512,256] -> SBUF [128, 4, 256] (din-chunk c at free index c).
    w_r = wpool.tile([128, DCH, KD], MMT)

    # Prewarm the ACT sqrt table set so the ~2.7us LoadActFuncSet overlaps
    # the x DMAs instead of sitting in the squash critical path.
    warm = small.tile([1, 1], F32, tag="warm")
    nc.scalar.activation(warm, one_one, ACT.Sqrt, bias=eps_t)

    MUL = mybir.AluOpType.mult
    vstate = {}

    def emit_front(rep):
        # x DMAs + ONE stage-1 accumulation group for both batches.
        # Chunks 0-3 stream straight to the PE; the otherwise-idle GpSimd
        # engine folds chunks 4-7 pairwise (exact fp32 adds), cutting PE
        # stage-1 columns 8192 -> 6144 per iteration without gating the PE
        # start (the folds are the LAST rhs in the group, so they complete
        # while the PE chews the direct chunks).
        xtiles = []
        folds = []
        for b in range(B_LOC):
            xr = x[b].rearrange("(a p) d -> p a d", p=128)  # [128, NCH, DIN]
            xt = xpool.tile([128, NCH, DIN], MMT, tag="x")
            for c in range(NCH // CPD):
                nc.sync.dma_start(
                    out=xt[:, c * CPD:(c + 1) * CPD, :],
                    in_=xr[:, c * CPD:(c + 1) * CPD, :].bitcast(MMT),
                )
            xtiles.append(xt)
            if rep == 0 and b == 0:
                nc.sync.dma_start(
                    out=w_r,
                    in_=w.rearrange("(c p) d -> p c d", p=128).bitcast(MMT),
                )
            fold = fpool.tile([128, 2, DIN], MMT, tag="fold")
            for f in range(2):
                nc.gpsimd.tensor_add(
                    fold[:, f, :], xt[:, 4 + 2 * f, :], xt[:, 5 + 2 * f, :]
                )
            folds.append(fold)
        v_ps = ps_v.tile([2, DIN], F32, tag="v")
        for b in range(B_LOC):
            rhs_list = [xtiles[b][:, a, :] for a in range(4)] + [
                folds[b][:, f, :] for f in range(2)
            ]
            for i, rhs in enumerate(rhs_list):
                nc.tensor.matmul(
                    v_ps,
                    lhsT=ind[b],
                    rhs=rhs,
                    start=(b == 0 and i == 0),
                    stop=(b == B_LOC - 1 and i == len(rhs_list) - 1),
                )
        vstate[rep] = v_ps

    def emit_tail(rep):
        v_ps = vstate.pop(rep)
        # v -> SBUF; free-dim halves on DVE + ACT in parallel
        v_sb = small.tile([2, DIN], F32, tag="v_sb")
        nc.vector.tensor_copy(v_sb[:, :DIN // 2], v_ps[:, :DIN // 2])
        nc.scalar.copy(v_sb[:, DIN // 2:], v_ps[:, DIN // 2:])

        # transpose both batches at once: vt[:, dj, b] = v[b, dj*128:...]
        vt_ps = ps_t.tile([128, DCH, 2], F32, tag="vt")
        for dj in range(DCH):
            nc.tensor.matmul(
                vt_ps[:, dj, :],
                lhsT=v_sb[:, dj * 128:(dj + 1) * 128],
                rhs=eye2,
                start=True,
                stop=True,
            )
        vt_sb = small.tile([128, DCH, 2], MMT, tag="vt_sb")
        nc.vector.tensor_copy(vt_sb, vt_ps)

        # y[b] = v[b] @ W for both batches in one accumulation group
        y_ps = ps_y.tile([2, KD], F32, tag="y")
        for dj in range(DCH):
            nc.tensor.matmul(
                y_ps,
                lhsT=vt_sb[:, dj, :],
                rhs=w_r[:, dj, :],
                start=(dj == 0),
                stop=(dj == DCH - 1),
            )
        y_sb = small.tile([2, KD], F32, tag="y_sb")
        nc.vector.tensor_copy(y_sb, y_ps)

        # squash scale per batch: sc = sq / ((1+sq) * sqrt(sq+eps)),
        # entirely on DVE/ACT; applied to y BEFORE the broadcast so the PE
        # broadcast matmul uses a constant lhsT and never waits on this.
        ysq = small.tile([2, KD], F32, tag="ysq")
        sq = small.tile([2, 1], F32, tag="sq")
        nc.vector.scalar_tensor_tensor(
            ysq, y_sb, 1.0, y_sb, op0=MUL, op1=MUL, accum_out=sq
        )
        s0 = small.tile([2, 1], F32, tag="s0")
        nc.scalar.activation(s0, sq, ACT.Sqrt, bias=eps2)
        d2 = small.tile([2, 1], F32, tag="d2")
        nc.vector.scalar_tensor_tensor(
            d2, sq, 1.0, s0, op0=mybir.AluOpType.add, op1=MUL
        )
        d3 = small.tile([2, 1], F32, tag="d3")
        nc.vector.reciprocal(d3, d2)
        sc = small.tile([2, 1], F32, tag="sc")
        nc.vector.tensor_mul(sc, sq, d3)
        ysc = small.tile([2, KD], MMT, tag="ysc")
        nc.vector.tensor_scalar_mul(ysc, y_sb, sc)

        # broadcast to 16 capsules per batch with the constant 0/1 block
        o_ps = ps_o.tile([2 * NCAPS, KD], F32, tag="o")
        nc.tensor.matmul(o_ps, lhsT=blk, rhs=ysc, start=True, stop=True)
        o_sb = outp.tile([2 * NCAPS, KD], F32, tag="o_sb")
        nc.scalar.copy(o_sb, o_ps)
        nc.scalar.dma_start(out=o.rearrange("b k d -> (b k) d"), in_=o_sb)

    # software pipeline: tail(rep-1) is emitted after front(rep) so the
    # in-order engine queues interleave the tail's latency chain with the
    # next rep's bulk work.
    for rep in range(repeats):
        emit_front(rep)
        if rep >= 1:
            emit_tail(rep - 1)
    emit_tail(repeats - 1)


def _build_nc(repeats: int = 1):
    nc = bacc.Bacc(
        "TRN2",
        target_bir_lowering=False,
        debug=False,
        num_devices=N_CORES,
    )
    x = nc.dram_tensor("x", [B_LOC, N, DIN], F32, kind="ExternalInput")
    w = nc.dram_tensor("w", [DIN, KD], F32, kind="ExternalInput")
    o = nc.dram_tensor("o", [B_LOC, NCAPS, KD], F32, kind="ExternalOutput")
    with tile.TileContext(nc) as tc:
        _capsule_body(tc, x.ap(), w.ap(), o.ap(), repeats=repeats)
    nc.compile()
    return nc


class Runner:
    """Cached PJRT executor for the SPMD bass kernel (8 cores).

    Mirrors concourse.bass2jax.run_bass_via_pjrt's multi-core path, but
    keeps the jitted executable alive so repeated kernel() calls don't
    re-trace/re-lower.
    """

    def __init__(self, repeats: int = 1):
        import jax
        from jax.experimental.shard_map import shard_map
        from jax.sharding import Mesh, PartitionSpec

        from concourse import bass2jax

        bass2jax.install_neuronx_cc_hook()
        self.nc = _build_nc(repeats=repeats)
        nc = self.nc

        partition_name = (
            nc.partition_id_tensor.name if nc.partition_id_tensor else None
        )
        in_names, out_names, out_avals, zero_outs = [], [], [], []
        for alloc in nc.m.functions[0].allocations:
            if not isinstance(alloc, mybir.MemoryLocationSet):
                continue
            name = alloc.memorylocations[0].name
            if alloc.kind == "ExternalInput":
                if name != partition_name:
                    in_names.append(name)
            elif alloc.kind == "ExternalOutput":
                shape = tuple(alloc.tensor_shape)
                dtype = mybir.dt.np(alloc.dtype)
                out_names.append(name)
                out_avals.append(jax.core.ShapedArray(shape, dtype))
                zero_outs.append(np.zeros(shape, dtype))
        self.in_names = in_names
        self.out_names = out_names
        self.out_avals = out_avals
        self.zero_outs = zero_outs
        n_params = len(in_names)
        n_outs = len(out_avals)
        all_in_names = in_names + out_names
        if partition_name is not None:
            all_in_names.append(partition_name)

        def _body(*args):
            operands = list(args)
            if partition_name is not None:
                operands.append(bass2jax.partition_id_tensor())
            outs = bass2jax._bass_exec_p.bind(
                *operands,
                out_avals=tuple(out_avals),
                in_names=tuple(all_in_names),
                out_names=tuple(out_names),
                lowering_input_output_aliases=(),
                sim_require_finite=True,
                sim_require_nnan=True,
                nc=nc,
            )
            return tuple(outs)

        self._body = _body
        devices = jax.devices()[:N_CORES]
        assert len(devices) == N_CORES
        self.mesh = Mesh(np.asarray(devices), ("core",))
        in_specs = (PartitionSpec("core"),) * (n_params + n_outs)
        out_specs = (PartitionSpec("core"),) * n_outs
        self.jitted = jax.jit(
            shard_map(
                _body,
                mesh=self.mesh,
                in_specs=in_specs,
                out_specs=out_specs,
                check_rep=False,
            ),
            donate_argnums=tuple(range(n_params, n_params + n_outs)),
            keep_unused=True,
        )

    def concat_inputs(self, in_maps):
        return [
            np.concatenate([np.asarray(m[name]) for m in in_maps], axis=0)
            for name in self.in_names
        ]

    def concat_zeros(self):
        return [
            np.zeros((N_CORES * z.shape[0], *z.shape[1:]), z.dtype)
            for z in self.zero_outs
        ]

    def __call__(self, concat_in):
        out_arrs = self.jitted(*concat_in, *self.concat_zeros())
        return [np.asarray(a) for a in out_arrs]


_RUNNERS: dict = {}


def get_runner(repeats: int = 1) -> Runner:
    if repeats not in _RUNNERS:
        _RUNNERS[repeats] = Runner(repeats=repeats)
    return _RUNNERS[repeats]


def kernel(inputs: np.ndarray, W: np.ndarray):
    """Full-input entry point: shard over 8 cores, run, gather."""
    assert inputs.shape == (B, N, DIN) and W.shape == (DIN, KD)
    runner = get_runner(1)
    xf = np.ascontiguousarray(inputs, dtype=np.float32)
    wf = np.ascontiguousarray(W, dtype=np.float32)
    in_maps = [
        {"x": xf[c * B_LOC:(c + 1) * B_LOC], "w": wf} for c in range(N_CORES)
    ]
    outs = runner(runner.concat_inputs(in_maps))
    # single output "o": [N_CORES * B_LOC, NCAPS, KD] -> [B, NCAPS, KD]
    return outs[0].reshape(B, NCAPS, KD)
